# revision 33
# baseline (speedup 1.0000x reference)
"""Causal self-attention (B=1, S=4096, D=1024, 16 heads) on 8 trn2 NeuronCores.

Sharding: tensor-parallel over heads (2 heads per core). Each core computes
qkv projection for its head pair, causal attention, and a partial output
projection; the host sums the 8 partials and adds b_out.

Device kernel (per core, all matmuls in float32r, fp32 PSUM accumulation):
  The emission interleaves projection / output-projection work INTO the
  attention j-loop so the in-order PE stream has fill work during the
  QK->exp->PV dependency gaps (exp on ACT is the j-loop rate limiter at
  ~1038ns/tile vs PE's ~854ns).
  - phase 1 (chunk projections, split into per-m fill pieces): qT/kT/vT =
    w_shard.T @ xT; bias added on the DVE copy out of PSUM. Head B's q/k
    rows are relocated to partitions 0-63 via SBUF->SBUF DMA. V (natural
    layout) is built by PE-transposing vT k-tiles.
  - phase 2 (per 512-wide q chunk): scores^T = K_tile.T @ Q per head (K=64),
    exp on ACT (scale=1/8 fused; scores are bounded so no max-subtraction
    is needed), causal-mask multiply on diagonal tiles, PV with an appended
    ones-column ([V|1], M=65) so row 64 of the accumulator is the softmax
    denominator; reciprocal -> gpsimd partition-broadcast (directly from
    partition 64) -> normalize; head B's normalized ctx is DMA-relocated to
    partitions 64-127 so the out-projection runs K=128.
  - x loads prefetch 2 chunks ahead; all DMA issues go through the SP
    engine's HWDGE (Pool/SWDGE issue costs ~1us fixed per DMA and Pool is
    needed for the causal mask + broadcast).
"""
import sys

sys.path.insert(0, "/opt/trn_rl_repo")

from contextlib import ExitStack

import numpy as np

import concourse.tile as tile
from concourse import bacc, mybir
from concourse.alu_op_type import AluOpType
from concourse.masks import make_identity
from concourse.bass_utils import run_bass_kernel_spmd

D = 1024
N_CORES = 8
F32 = mybir.dt.float32
F32R = mybir.dt.float32r
AF = mybir.ActivationFunctionType

QC = 512  # q-chunk width
KT = 128  # k-tile width


def build_program(S: int = 4096, repeat: int = 1):
    nqc = S // QC

    nc = bacc.Bacc(None)
    xT = nc.declare_dram_parameter("xT", [D, S], F32R, isOutput=False)
    w_sh = nc.declare_dram_parameter("w_sh", [D, 384], F32R, isOutput=False)
    b_sh = nc.declare_dram_parameter("b_sh", [384], F32, isOutput=False)
    w_o = nc.declare_dram_parameter("w_o", [128, D], F32R, isOutput=False)
    outp = nc.declare_dram_parameter("outp", [S, D], F32, isOutput=True)

    with tile.TileContext(nc) as tc, ExitStack() as ctx:
        consts = ctx.enter_context(tc.tile_pool(name="consts", bufs=1))
        big = ctx.enter_context(tc.tile_pool(name="big", bufs=1))
        xpool = ctx.enter_context(tc.tile_pool(name="xp", bufs=2))
        vtpool = ctx.enter_context(tc.tile_pool(name="vt", bufs=2))
        stpool = ctx.enter_context(tc.tile_pool(name="st", bufs=3))
        apool = ctx.enter_context(tc.tile_pool(name="at", bufs=8))
        npool = ctx.enter_context(tc.tile_pool(name="nrm", bufs=1))
        opool = ctx.enter_context(tc.tile_pool(name="ot", bufs=3))
        psS = ctx.enter_context(tc.tile_pool(name="psS", bufs=2, space="PSUM"))
        psCA = ctx.enter_context(tc.tile_pool(name="psCA", bufs=1, space="PSUM"))
        psCB = ctx.enter_context(tc.tile_pool(name="psCB", bufs=1, space="PSUM"))
        psP = ctx.enter_context(tc.tile_pool(name="psP", bufs=1, space="PSUM"))
        psQ = ctx.enter_context(tc.tile_pool(name="psQ", bufs=1, space="PSUM"))

        # ---- constants
        ident_f = consts.tile([128, 128], F32)
        make_identity(nc, ident_f[:])
        ident = consts.tile([128, 128], F32R)
        nc.vector.tensor_copy(ident[:], ident_f[:])

        ones_f = consts.tile([128, 8], F32)
        nc.gpsimd.memset(ones_f[:], 1.0)

        def emit_xload(n):
            xt = xpool.tile([128, 8, QC], F32R, tag="xt", name="xt")
            src = xT.rearrange("(t p) s -> p t s", p=128)
            nc.sync.dma_start(xt[:, 0:4, :], src[:, 0:4, n * QC:(n + 1) * QC])
            nc.sync.dma_start(xt[:, 4:8, :], src[:, 4:8, n * QC:(n + 1) * QC])
            return xt

        # load order: first projection's deps (biases, w_sb m=0 slice, x
        # chunk 0) go first so proj(0) starts ~5us earlier
        w_sb = consts.tile([128, 8, 384], F32R)
        biases = consts.tile([128, 3], F32)
        nc.sync.dma_start(biases[:], b_sh.rearrange("(m p) -> p m", p=128))
        w_src = w_sh.rearrange("(t p) m -> p t m", p=128)
        nc.sync.dma_start(w_sb[:, :, 0:128], w_src[:, :, 0:128])
        xts_pre = {0: emit_xload(0)}
        for m in range(1, 3):
            nc.sync.dma_start(
                w_sb[:, :, m * 128:(m + 1) * 128], w_src[:, :, m * 128:(m + 1) * 128]
            )
        xts_pre[1] = emit_xload(1)
        w_o_sb = consts.tile([128, D], F32R)
        nc.sync.dma_start(w_o_sb[:], w_o[:])

        # per-chunk projection tiles (separate tags so attention on chunk c
        # only depends on projections of chunks <= c)
        qk_t = [
            big.tile([64, 2, 2, QC], F32R, tag=f"qk{n}", name=f"qk{n}")
            for n in range(nqc)
        ]
        v_t = [
            big.tile([128, 4, 130], F32R, tag=f"v{n}", name=f"v{n}")
            for n in range(nqc)
        ]
        for n in range(nqc):
            nc.vector.tensor_copy(
                v_t[n][:].rearrange("p t (g c) -> p t g c", g=2)[:, :, :, 64:65],
                ones_f[:].rearrange("p (t g o) -> p t g o", g=2, o=1),
            )

        for _rep in range(repeat):
            def proj_quanta(n, xt):
                # fill quanta at single-matmul granularity (~213ns each) so
                # the in-order PE stream interleaves finely with the
                # attention pipeline and never starves ACT.
                stage_box = []
                ps_box = {}

                def mk_mm(m, t):
                    def f():
                        if t == 0:
                            ps_box[m] = psP.tile(
                                [128, QC], F32, tag="proj", name="ps"
                            )
                        nc.tensor.matmul(
                            ps_box[m][:],
                            w_sb[:, t, m * 128:(m + 1) * 128],
                            xt[:, t, :],
                            start=(t == 0),
                            stop=(t == 7),
                        )
                    return f

                def mk_epi(m):
                    def f():
                        ps = ps_box[m]
                        if m < 2:
                            if not stage_box:
                                stage_box.append(
                                    stpool.tile([128, 2, QC], F32R, name="stage")
                                )
                            stage = stage_box[0]
                            nc.vector.tensor_scalar_add(
                                qk_t[n][:, 0, m, :], ps[0:64, :],
                                biases[0:64, m:m + 1],
                            )
                            nc.vector.tensor_scalar_add(
                                stage[64:128, m, :], ps[64:128, :],
                                biases[64:128, m:m + 1],
                            )
                            if m == 1:
                                nc.gpsimd.dma_start(
                                    qk_t[n][:, 1, :, :], stage[64:128, :, :]
                                )
                        else:
                            vt_c = vtpool.tile([128, QC], F32R)
                            nc.vector.tensor_scalar_add(
                                vt_c[:], ps[:], biases[:, 2:3]
                            )
                            tr = psP.tile(
                                [128, 4, 128], F32R, tag="proj", name="tr"
                            )
                            for s in range(4):
                                nc.tensor.transpose(
                                    tr[:, s, :], vt_c[:, s * 128:(s + 1) * 128],
                                    ident[:],
                                )
                            nc.vector.tensor_copy(
                                v_t[n][:].rearrange(
                                    "p t (g c) -> p t g c", g=2)[:, :, :, 0:64],
                                tr[:].rearrange("p t (g c) -> p t g c", g=2),
                            )
                    return f

                out = []
                for m in range(3):
                    out.extend(mk_mm(m, t) for t in range(8))
                    out.append(mk_epi(m))
                return out

            def emit_jloop(c, fill):
                ctxA = psCA.tile([65, QC], F32, tag="ctxA")
                ctxB = psCB.tile([65, QC], F32, tag="ctxB")
                jmax = 4 * (c + 1)
                done = 0

                def emit_pv(j):
                    # PV for tile j is deferred one iteration so the in-order
                    # PE stream executes it (plus fill) while exp(j+1) runs on
                    # ACT: the QK->exp->PV latency never exposes on PE.
                    p = j - 4 * c
                    # fp32r matmul drops to 4 cycles/row below a 256-wide
                    # moving dim; widen the last diagonal tile (the extra
                    # columns are fully masked anyway)
                    off = min(max(0, p) * KT, QC - 256)
                    n_j, s_j = j // 4, j % 4
                    at = ats[j]
                    first, last = (j == 0), (j == jmax - 1)
                    nc.tensor.matmul(
                        ctxA[:, off:], v_t[n_j][:, s_j, 0:65], at[:, 0, off:],
                        start=first, stop=last,
                    )
                    nc.tensor.matmul(
                        ctxB[:, off:], v_t[n_j][:, s_j, 65:130], at[:, 1, off:],
                        start=first, stop=last,
                    )

                ats = {}
                for j in range(jmax):
                    p = j - 4 * c
                    off = min(max(0, p) * KT, QC - 256)
                    n_j, s_j = j // 4, j % 4
                    sc = psS.tile([128, 2, QC], F32, tag="sc")
                    for h in range(2):
                        nc.tensor.matmul(
                            sc[:, h, off:],
                            qk_t[n_j][:, h, 1, s_j * KT:(s_j + 1) * KT],
                            qk_t[c][:, h, 0, off:],
                            start=True, stop=True,
                        )
                    at = apool.tile([128, 2, QC], F32R)
                    nc.scalar.activation(
                        at[:, :, off:], sc[:, :, off:], AF.Exp, scale=0.125
                    )
                    if p >= 0:
                        # zero the upper-triangular wedge in place (both heads
                        # in one op; head dim has pattern step 0):
                        # keep iff (off + q_local) - k - 128*p >= 0
                        nc.gpsimd.affine_select(
                            out=at[:, :, off:], in_=at[:, :, off:],
                            pattern=[[0, 2], [1, QC - off]],
                            compare_op=AluOpType.is_ge,
                            fill=0.0, base=off - KT * p, channel_multiplier=-1,
                        )
                    ats[j] = at
                    if j > 0:
                        emit_pv(j - 1)
                        del ats[j - 1]
                    want = (len(fill) * (j + 1)) // jmax
                    while done < want:
                        fill[done]()
                        done += 1
                emit_pv(jmax - 1)
                return ctxA, ctxB

            def emit_norm_pre(c, ctxA, ctxB):
                recip = npool.tile([65, 2, QC], F32, tag="recip")
                nc.vector.reciprocal(recip[64:65, 0, :], ctxA[64:65, :])
                nc.vector.reciprocal(recip[64:65, 1, :], ctxB[64:65, :])
                scr = npool.tile([1, 2, QC], F32, tag="scr")
                nc.gpsimd.dma_start(scr[:], recip[64:65, :, :])
                bc = npool.tile([64, 2, QC], F32, tag="bc")
                nc.gpsimd.partition_broadcast(bc[:], scr[:])
                ctxn = npool.tile([128, QC], F32R, tag="ctxn")
                nc.vector.tensor_mul(ctxn[0:64, :], ctxA[0:64, :], bc[:, 0, :])
                ctxnB = npool.tile([64, QC], F32R, tag="ctxnB")
                nc.vector.tensor_mul(ctxnB[:], ctxB[0:64, :], bc[:, 1, :])
                nc.gpsimd.dma_start(ctxn[64:128, :], ctxnB[:])
                return ctxn

            def outproj_quanta(c, ctxn, tail=False):
                ot_box = {}

                def mk(s, half):
                    def f():
                        if half == 0:
                            ot_box[s] = opool.tile([128, D], F32, name="ot")
                        ot = ot_box[s]
                        if tail and (2 * s + half) % 2 == 1:
                            # outside the j-loop the scores ring is idle; use
                            # it as a second buffer so copies overlap matmuls
                            op = psS.tile([128, QC], F32, tag="sc", name="op")
                        else:
                            op = psQ.tile([128, QC], F32, tag="oproj", name="op")
                        nc.tensor.matmul(
                            op[:],
                            ctxn[:, s * 128:(s + 1) * 128],
                            w_o_sb[:, half * QC:(half + 1) * QC],
                            start=True, stop=True,
                        )
                        nc.vector.tensor_copy(
                            ot[:, half * QC:(half + 1) * QC], op[:]
                        )
                        if half == 1:
                            row = c * QC + s * 128
                            nc.sync.dma_start(outp[row:row + 128, :], ot[:])
                    return f
                return [mk(s, h) for s in range(4) for h in range(2)]

            def merge(a, b):
                # proportional interleave of two quanta lists
                out, ia, ib = [], 0, 0
                while ia < len(a) or ib < len(b):
                    if ib >= len(b) or (
                        ia < len(a) and ia * len(b) <= ib * len(a)
                    ):
                        out.append(a[ia])
                        ia += 1
                    else:
                        out.append(b[ib])
                        ib += 1
                return out

            # ---- prologue (first rep reuses the preloaded x chunks)
            if _rep == 0:
                xts = dict(xts_pre)
            else:
                xts = {0: emit_xload(0)}
                if nqc > 1:
                    xts[1] = emit_xload(1)
            for piece in proj_quanta(0, xts[0]):
                piece()

            pending_out = []
            for c in range(nqc):
                fill = []
                if c + 2 < nqc:
                    def mk_load(n):
                        def f():
                            xts[n] = emit_xload(n)
                        return f
                    fill.append(mk_load(c + 2))
                projp = proj_quanta(c + 1, xts[c + 1]) if c + 1 < nqc else []
                fill += merge(projp, pending_out)
                ctxA, ctxB = emit_jloop(c, fill)
                ctxn = emit_norm_pre(c, ctxA, ctxB)
                pending_out = outproj_quanta(c, ctxn, tail=(c == nqc - 1))
            for piece in pending_out:
                piece()
    nc.compile()
    return nc


_PROGRAM_CACHE: dict = {}


def _get_program(S: int):
    if S not in _PROGRAM_CACHE:
        _PROGRAM_CACHE[S] = build_program(S)
    return _PROGRAM_CACHE[S]


def make_in_maps(x, w_qkv, b_qkv, w_out):
    x = np.asarray(x, dtype=np.float32)
    w_qkv = np.asarray(w_qkv, dtype=np.float32)
    b_qkv = np.asarray(b_qkv, dtype=np.float32)
    w_out = np.asarray(w_out, dtype=np.float32)
    S = x.shape[1]
    xT = np.ascontiguousarray(x.reshape(S, D).T)
    in_maps = []
    for c in range(N_CORES):
        lo, hi = 128 * c, 128 * (c + 1)
        w_shard = np.ascontiguousarray(
            np.concatenate(
                [w_qkv[:, lo:hi], w_qkv[:, D + lo:D + hi], w_qkv[:, 2 * D + lo:2 * D + hi]],
                axis=1,
            )
        )
        b_shard = np.concatenate(
            [b_qkv[lo:hi], b_qkv[D + lo:D + hi], b_qkv[2 * D + lo:2 * D + hi]]
        )
        w_o_shard = np.ascontiguousarray(w_out[lo:hi, :])
        in_maps.append(
            {"xT": xT, "w_sh": w_shard, "b_sh": b_shard, "w_o": w_o_shard}
        )
    return in_maps


def kernel(x, w_qkv, b_qkv, w_out, b_out):
    x = np.asarray(x, dtype=np.float32)
    b_out = np.asarray(b_out, dtype=np.float32)
    B, S, _ = x.shape
    in_maps = make_in_maps(x, w_qkv, b_qkv, w_out)
    nc = _get_program(S)
    res = run_bass_kernel_spmd(nc, in_maps, list(range(N_CORES))).results
    out = res[0]["outp"].copy()
    for c in range(1, N_CORES):
        out += res[c]["outp"]
    out += b_out
    return out.reshape(B, S, D)


# revision 34
# speedup vs baseline: 1.3230x; 1.3230x over previous
"""Causal self-attention (B=1, S=4096, D=1024, 16 heads) on 8 trn2 NeuronCores.

Sharding: tensor-parallel over heads (2 heads per core). Each core computes
qkv projection for its head pair, causal attention, and a partial output
projection; the host sums the 8 partials and adds b_out.

Device kernel (per core, all matmuls in float32r, fp32 PSUM accumulation):
  Emission interleaves projection / output-projection work INTO the
  attention j-loop at single-matmul granularity (~213ns quanta) so the
  in-order PE stream has fill during the QK->exp->PV dependency gaps (exp
  on ACT is the j-loop rate limiter at ~1038ns/tile vs PE's ~854ns), and
  each PV is deferred one j-tile so exp latency never exposes on PE.
  - projections (per 512-seq chunk, split into per-m quanta): qT/kT/vT =
    w_shard.T @ xT; bias added on the DVE copy out of PSUM. Head B's q/k
    rows are relocated to partitions 0-63 via SBUF->SBUF DMA (SWDGE). V
    (natural layout) is built by PE-transposing vT k-tiles.
  - attention (per 512-wide q chunk): scores^T = K_tile.T @ Q per head
    (K=64), exp on ACT (scale=1/8 fused; scores are bounded so no
    max-subtraction is needed), causal-mask multiply on diagonal tiles, PV
    with an appended ones-column ([V|1], M=65) so row 64 of the accumulator
    is the softmax denominator; reciprocal -> SWDGE row hop to partition 0
    -> gpsimd partition-broadcast -> normalize; head B's normalized ctx is
    DMA-relocated to partitions 64-127 so the out-projection runs K=128.
  - diagonal j-tiles are widened to a >=256 moving dim (fp32r matmul drops
    to 4 cycles/row below 256); the extra columns are fully masked.
  - x loads (HWDGE on SP) prefetch 2 chunks ahead; the first projection's
    dependencies (biases, w m=0, x chunk 0) are loaded first.
  - PSUM: scores 2x2 banks, ctxA/ctxB 1+1, proj accum + transposes 1,
    out-proj 1; the tail out-projection alternates into the then-idle
    scores ring so copies overlap matmuls.
"""
import sys

sys.path.insert(0, "/opt/trn_rl_repo")

from contextlib import ExitStack

import numpy as np

import concourse.tile as tile
from concourse import bacc, mybir
from concourse.alu_op_type import AluOpType
from concourse.masks import make_identity
from concourse.bass_utils import run_bass_kernel_spmd

D = 1024
N_CORES = 8
F32 = mybir.dt.float32
F32R = mybir.dt.float32r
AF = mybir.ActivationFunctionType

QC = 512  # q-chunk width
KT = 128  # k-tile width


def build_program(S: int = 4096, repeat: int = 1):
    nqc = S // QC

    nc = bacc.Bacc(None)
    xT = nc.declare_dram_parameter("xT", [D, S], F32R, isOutput=False)
    w_sh = nc.declare_dram_parameter("w_sh", [D, 384], F32R, isOutput=False)
    b_sh = nc.declare_dram_parameter("b_sh", [384], F32, isOutput=False)
    w_o = nc.declare_dram_parameter("w_o", [128, D], F32R, isOutput=False)
    outp = nc.declare_dram_parameter("outp", [S, D], F32, isOutput=True)

    with tile.TileContext(nc) as tc, ExitStack() as ctx:
        consts = ctx.enter_context(tc.tile_pool(name="consts", bufs=1))
        big = ctx.enter_context(tc.tile_pool(name="big", bufs=1))
        xpool = ctx.enter_context(tc.tile_pool(name="xp", bufs=2))
        vtpool = ctx.enter_context(tc.tile_pool(name="vt", bufs=2))
        stpool = ctx.enter_context(tc.tile_pool(name="st", bufs=3))
        apool = ctx.enter_context(tc.tile_pool(name="at", bufs=8))
        npool = ctx.enter_context(tc.tile_pool(name="nrm", bufs=1))
        opool = ctx.enter_context(tc.tile_pool(name="ot", bufs=3))
        psS = ctx.enter_context(tc.tile_pool(name="psS", bufs=2, space="PSUM"))
        psCA = ctx.enter_context(tc.tile_pool(name="psCA", bufs=1, space="PSUM"))
        psCB = ctx.enter_context(tc.tile_pool(name="psCB", bufs=1, space="PSUM"))
        psP = ctx.enter_context(tc.tile_pool(name="psP", bufs=1, space="PSUM"))
        psQ = ctx.enter_context(tc.tile_pool(name="psQ", bufs=1, space="PSUM"))

        # ---- constants
        ident_f = consts.tile([128, 128], F32)
        make_identity(nc, ident_f[:])
        ident = consts.tile([128, 128], F32R)
        nc.vector.tensor_copy(ident[:], ident_f[:])

        ones_f = consts.tile([128, 8], F32)
        nc.gpsimd.memset(ones_f[:], 1.0)

        def emit_xload(n):
            xt = xpool.tile([128, 8, QC], F32R, tag="xt", name="xt")
            src = xT.rearrange("(t p) s -> p t s", p=128)
            nc.sync.dma_start(xt[:, 0:4, :], src[:, 0:4, n * QC:(n + 1) * QC])
            nc.sync.dma_start(xt[:, 4:8, :], src[:, 4:8, n * QC:(n + 1) * QC])
            return xt

        # load order: first projection's deps (biases, w_sb m=0 slice, x
        # chunk 0) go first so proj(0) starts ~5us earlier
        w_sb = consts.tile([128, 8, 384], F32R)
        biases = consts.tile([128, 3], F32)
        nc.sync.dma_start(biases[:], b_sh.rearrange("(m p) -> p m", p=128))
        w_src = w_sh.rearrange("(t p) m -> p t m", p=128)
        nc.sync.dma_start(w_sb[:, :, 0:128], w_src[:, :, 0:128])
        xts_pre = {0: emit_xload(0)}
        for m in range(1, 3):
            nc.sync.dma_start(
                w_sb[:, :, m * 128:(m + 1) * 128], w_src[:, :, m * 128:(m + 1) * 128]
            )
        xts_pre[1] = emit_xload(1)
        w_o_sb = consts.tile([128, D], F32R)
        nc.sync.dma_start(w_o_sb[:], w_o[:])

        # per-chunk projection tiles (separate tags so attention on chunk c
        # only depends on projections of chunks <= c)
        qk_t = [
            big.tile([64, 2, 2, QC], F32R, tag=f"qk{n}", name=f"qk{n}")
            for n in range(nqc)
        ]
        v_t = [
            big.tile([128, 4, 130], F32R, tag=f"v{n}", name=f"v{n}")
            for n in range(nqc)
        ]
        for n in range(nqc):
            nc.vector.tensor_copy(
                v_t[n][:].rearrange("p t (g c) -> p t g c", g=2)[:, :, :, 64:65],
                ones_f[:].rearrange("p (t g o) -> p t g o", g=2, o=1),
            )

        for _rep in range(repeat):
            def proj_quanta(n, xt):
                # fill quanta at single-matmul granularity (~213ns each) so
                # the in-order PE stream interleaves finely with the
                # attention pipeline and never starves ACT.
                stage_box = []
                ps_box = {}

                def mk_mm(m, t):
                    def f():
                        if t == 0:
                            ps_box[m] = psP.tile(
                                [128, QC], F32, tag="proj", name="ps"
                            )
                        nc.tensor.matmul(
                            ps_box[m][:],
                            w_sb[:, t, m * 128:(m + 1) * 128],
                            xt[:, t, :],
                            start=(t == 0),
                            stop=(t == 7),
                        )
                    return f

                def mk_epi(m):
                    def f():
                        ps = ps_box[m]
                        if m < 2:
                            if not stage_box:
                                stage_box.append(
                                    stpool.tile([128, 2, QC], F32R, name="stage")
                                )
                            stage = stage_box[0]
                            nc.vector.tensor_scalar_add(
                                qk_t[n][:, 0, m, :], ps[0:64, :],
                                biases[0:64, m:m + 1],
                            )
                            nc.vector.tensor_scalar_add(
                                stage[64:128, m, :], ps[64:128, :],
                                biases[64:128, m:m + 1],
                            )
                            if m == 1:
                                nc.gpsimd.dma_start(
                                    qk_t[n][:, 1, :, :], stage[64:128, :, :]
                                )
                        else:
                            vt_c = vtpool.tile([128, QC], F32R)
                            nc.vector.tensor_scalar_add(
                                vt_c[:], ps[:], biases[:, 2:3]
                            )
                            tr = psP.tile(
                                [128, 4, 128], F32R, tag="proj", name="tr"
                            )
                            for s in range(4):
                                nc.tensor.transpose(
                                    tr[:, s, :], vt_c[:, s * 128:(s + 1) * 128],
                                    ident[:],
                                )
                            nc.vector.tensor_copy(
                                v_t[n][:].rearrange(
                                    "p t (g c) -> p t g c", g=2)[:, :, :, 0:64],
                                tr[:].rearrange("p t (g c) -> p t g c", g=2),
                            )
                    return f

                out = []
                for m in range(3):
                    out.extend(mk_mm(m, t) for t in range(8))
                    out.append(mk_epi(m))
                return out

            def emit_jloop(c, fill):
                ctxA = psCA.tile([65, QC], F32, tag="ctxA")
                ctxB = psCB.tile([65, QC], F32, tag="ctxB")
                jmax = 4 * (c + 1)
                done = 0

                def emit_pv(j):
                    # PV for tile j is deferred one iteration so the in-order
                    # PE stream executes it (plus fill) while exp(j+1) runs on
                    # ACT: the QK->exp->PV latency never exposes on PE.
                    p = j - 4 * c
                    # fp32r matmul drops to 4 cycles/row below a 256-wide
                    # moving dim; widen the last diagonal tile (the extra
                    # columns are fully masked anyway)
                    off = min(max(0, p) * KT, QC - 256)
                    n_j, s_j = j // 4, j % 4
                    at = ats[j]
                    first, last = (j == 0), (j == jmax - 1)
                    nc.tensor.matmul(
                        ctxA[:, off:], v_t[n_j][:, s_j, 0:65], at[:, 0, off:],
                        start=first, stop=last,
                    )
                    nc.tensor.matmul(
                        ctxB[:, off:], v_t[n_j][:, s_j, 65:130], at[:, 1, off:],
                        start=first, stop=last,
                    )

                ats = {}
                for j in range(jmax):
                    p = j - 4 * c
                    off = min(max(0, p) * KT, QC - 256)
                    n_j, s_j = j // 4, j % 4
                    sc = psS.tile([128, 2, QC], F32, tag="sc")
                    for h in range(2):
                        nc.tensor.matmul(
                            sc[:, h, off:],
                            qk_t[n_j][:, h, 1, s_j * KT:(s_j + 1) * KT],
                            qk_t[c][:, h, 0, off:],
                            start=True, stop=True,
                        )
                    at = apool.tile([128, 2, QC], F32R)
                    nc.scalar.activation(
                        at[:, :, off:], sc[:, :, off:], AF.Exp, scale=0.125
                    )
                    if p >= 0:
                        # zero the upper-triangular wedge in place (both heads
                        # in one op; head dim has pattern step 0):
                        # keep iff (off + q_local) - k - 128*p >= 0
                        nc.gpsimd.affine_select(
                            out=at[:, :, off:], in_=at[:, :, off:],
                            pattern=[[0, 2], [1, QC - off]],
                            compare_op=AluOpType.is_ge,
                            fill=0.0, base=off - KT * p, channel_multiplier=-1,
                        )
                    ats[j] = at
                    if j > 0:
                        emit_pv(j - 1)
                        del ats[j - 1]
                    want = (len(fill) * (j + 1)) // jmax
                    while done < want:
                        fill[done]()
                        done += 1
                emit_pv(jmax - 1)
                return ctxA, ctxB

            def emit_norm_pre(c, ctxA, ctxB):
                recip = npool.tile([65, 2, QC], F32, tag="recip")
                nc.vector.reciprocal(recip[64:65, 0, :], ctxA[64:65, :])
                nc.vector.reciprocal(recip[64:65, 1, :], ctxB[64:65, :])
                scr = npool.tile([1, 2, QC], F32, tag="scr")
                nc.gpsimd.dma_start(scr[:], recip[64:65, :, :])
                bc = npool.tile([64, 2, QC], F32, tag="bc")
                nc.gpsimd.partition_broadcast(bc[:], scr[:])
                ctxn = npool.tile([128, QC], F32R, tag="ctxn")
                nc.vector.tensor_mul(ctxn[0:64, :], ctxA[0:64, :], bc[:, 0, :])
                ctxnB = npool.tile([64, QC], F32R, tag="ctxnB")
                nc.vector.tensor_mul(ctxnB[:], ctxB[0:64, :], bc[:, 1, :])
                nc.gpsimd.dma_start(ctxn[64:128, :], ctxnB[:])
                return ctxn

            def outproj_quanta(c, ctxn, tail=False):
                ot_box = {}

                def mk(s, half):
                    def f():
                        if half == 0:
                            ot_box[s] = opool.tile([128, D], F32, name="ot")
                        ot = ot_box[s]
                        if tail and (2 * s + half) % 2 == 1:
                            # outside the j-loop the scores ring is idle; use
                            # it as a second buffer so copies overlap matmuls
                            op = psS.tile([128, QC], F32, tag="sc", name="op")
                        else:
                            op = psQ.tile([128, QC], F32, tag="oproj", name="op")
                        nc.tensor.matmul(
                            op[:],
                            ctxn[:, s * 128:(s + 1) * 128],
                            w_o_sb[:, half * QC:(half + 1) * QC],
                            start=True, stop=True,
                        )
                        nc.vector.tensor_copy(
                            ot[:, half * QC:(half + 1) * QC], op[:]
                        )
                        if half == 1:
                            row = c * QC + s * 128
                            nc.sync.dma_start(outp[row:row + 128, :], ot[:])
                    return f
                return [mk(s, h) for s in range(4) for h in range(2)]

            def merge(a, b):
                # proportional interleave of two quanta lists
                out, ia, ib = [], 0, 0
                while ia < len(a) or ib < len(b):
                    if ib >= len(b) or (
                        ia < len(a) and ia * len(b) <= ib * len(a)
                    ):
                        out.append(a[ia])
                        ia += 1
                    else:
                        out.append(b[ib])
                        ib += 1
                return out

            # ---- prologue (first rep reuses the preloaded x chunks)
            if _rep == 0:
                xts = dict(xts_pre)
            else:
                xts = {0: emit_xload(0)}
                if nqc > 1:
                    xts[1] = emit_xload(1)
            for piece in proj_quanta(0, xts[0]):
                piece()

            pending_out = []
            for c in range(nqc):
                fill = []
                if c + 2 < nqc:
                    def mk_load(n):
                        def f():
                            xts[n] = emit_xload(n)
                        return f
                    fill.append(mk_load(c + 2))
                projp = proj_quanta(c + 1, xts[c + 1]) if c + 1 < nqc else []
                fill += merge(projp, pending_out)
                ctxA, ctxB = emit_jloop(c, fill)
                ctxn = emit_norm_pre(c, ctxA, ctxB)
                pending_out = outproj_quanta(c, ctxn, tail=(c == nqc - 1))
            for piece in pending_out:
                piece()
    nc.compile()
    return nc


_PROGRAM_CACHE: dict = {}


def _get_program(S: int):
    if S not in _PROGRAM_CACHE:
        _PROGRAM_CACHE[S] = build_program(S)
    return _PROGRAM_CACHE[S]


def make_in_maps(x, w_qkv, b_qkv, w_out):
    x = np.asarray(x, dtype=np.float32)
    w_qkv = np.asarray(w_qkv, dtype=np.float32)
    b_qkv = np.asarray(b_qkv, dtype=np.float32)
    w_out = np.asarray(w_out, dtype=np.float32)
    S = x.shape[1]
    xT = np.ascontiguousarray(x.reshape(S, D).T)
    in_maps = []
    for c in range(N_CORES):
        lo, hi = 128 * c, 128 * (c + 1)
        w_shard = np.ascontiguousarray(
            np.concatenate(
                [w_qkv[:, lo:hi], w_qkv[:, D + lo:D + hi], w_qkv[:, 2 * D + lo:2 * D + hi]],
                axis=1,
            )
        )
        b_shard = np.concatenate(
            [b_qkv[lo:hi], b_qkv[D + lo:D + hi], b_qkv[2 * D + lo:2 * D + hi]]
        )
        w_o_shard = np.ascontiguousarray(w_out[lo:hi, :])
        in_maps.append(
            {"xT": xT, "w_sh": w_shard, "b_sh": b_shard, "w_o": w_o_shard}
        )
    return in_maps


def kernel(x, w_qkv, b_qkv, w_out, b_out):
    x = np.asarray(x, dtype=np.float32)
    b_out = np.asarray(b_out, dtype=np.float32)
    B, S, _ = x.shape
    in_maps = make_in_maps(x, w_qkv, b_qkv, w_out)
    nc = _get_program(S)
    res = run_bass_kernel_spmd(nc, in_maps, list(range(N_CORES))).results
    out = res[0]["outp"].copy()
    for c in range(1, N_CORES):
        out += res[c]["outp"]
    out += b_out
    return out.reshape(B, S, D)
